# revision 5
# baseline (speedup 1.0000x reference)
"""Causal single-head self-attention on 8 Trainium2 NeuronCores.

Problem: x [4, 4096, 1024], W_q/W_k/W_v [1024, 64] (+biases) ->
softmax(causal(QK^T / 8)) @ V, output [4, 4096, 64] fp32.

Sharding: data-parallel over batch (2 cores per batch). Within a pair,
k-tiles (128 rows) are parity-striped: core h in {0,1} owns k-tiles
{h, h+2, h+4, ...} of its batch. Softmax uses the no-max formulation
(scores are O(1) here), so per-core partial numerators/denominators are
linear and combine with a pairwise ReduceScatter(add). Every core runs
the IDENTICAL program (single SPMD NEFF); per-core differences (batch,
parity) are carried entirely by input data:
  - xts: x[b]^T with columns parity-split ([parity-0 k-tiles | parity-1])
    - shared by both cores of a pair
  - masks: causal-edge masks baked for the core's parity
  - hsel: register offset (h*2048) selecting the core's K/V half

Compute layout (per core):
  - All matmul operands are fp16 (full PE rate + fast weight load);
    accumulation stays fp32 in PSUM, softmax partials stay fp32.
  - Projections contract D=1024 on partitions: Q^T/[K|V]^T tiles come out
    of the PE as [features, rows].
  - Scores are computed transposed, S^T[k, q], so the softmax sum over k
    and the A@V contraction both sit on the partition axis and reduce via
    matmuls. The ones column folded into [V | 1] yields the softmax
    denominator in the same accumulation.
  - Causal masking: additive -1e30 on the two diagonal-crossing k-tiles
    per q-slot (DVE add on PSUM before the exp).
  - Transposes (V chunks, finalize) are normal-mode matmuls against an
    identity rhs: out = lhsT.T @ I.
  - Finalize: transpose [65, 128] chunks of the pair-reduced partials,
    DVE reciprocal of the denominator row, tensor_scalar multiply,
    DMA out in [q, 64] layout.
"""

import numpy as np

import concourse.bass as bass
import concourse.mybir as mybir
import concourse.tile as tile
from concourse import bacc
from concourse.bass_utils import run_bass_kernel_spmd

B, S, DM, H = 4, 4096, 1024, 64
P = 128
QW = 512            # q-tile width
NQ = S // QW        # 8 q-slots
DSL = DM // P       # 8 d-slices
KL = 16             # local (own-parity) k-tiles of 128
HALF = S // 2       # 2048
GROUPS = [[0, 1], [2, 3], [4, 5], [6, 7]]

f32 = mybir.dt.float32
f16 = mybir.dt.float16
u32 = mybir.dt.uint32
Act = mybir.ActivationFunctionType
Alu = mybir.AluOpType
NEG = -1.0e30


def build_nc():
    nc = bacc.Bacc("TRN2", target_bir_lowering=False, debug=False, num_devices=8)

    xts = nc.declare_dram_parameter("xts", [DM, S], f16, isOutput=False)
    wq = nc.declare_dram_parameter("wq", [DM, H], f16, isOutput=False)
    wkv = nc.declare_dram_parameter("wkv", [DM, 2 * H], f16, isOutput=False)
    bq = nc.declare_dram_parameter("bq", [H, 1], f32, isOutput=False)
    bkv = nc.declare_dram_parameter("bkv", [2 * H, 1], f32, isOutput=False)
    masks = nc.declare_dram_parameter("masks", [P, 2, QW], f32, isOutput=False)
    id2 = nc.declare_dram_parameter("id2", [P, H], f16, isOutput=False)
    idf = nc.declare_dram_parameter("idf", [P, P], f16, isOutput=False)
    ones = nc.declare_dram_parameter("ones", [P, KL], f16, isOutput=False)
    hsel = nc.declare_dram_parameter("hsel", [1, 1], u32, isOutput=False)
    out = nc.declare_dram_parameter("out", [HALF, H], f32, isOutput=True)

    with tile.TileContext(nc) as tc:
        with (
            tc.tile_pool(name="const", bufs=1) as const,
            tc.tile_pool(name="big", bufs=1) as big,
            tc.tile_pool(name="dram", bufs=1, space="DRAM") as dram,
        ):
            t_wq = const.tile([P, DSL, H], f16)
            nc.scalar.dma_start(
                out=t_wq[:, :, :],
                in_=wq[:, :].rearrange("(dsl p) hh -> p dsl hh", p=P),
            )
            t_wkv = const.tile([P, DSL, 2 * H], f16)
            nc.scalar.dma_start(
                out=t_wkv[:, :, :],
                in_=wkv[:, :].rearrange("(dsl p) hh -> p dsl hh", p=P),
            )
            t_bq = const.tile([H, 1], f32)
            nc.scalar.dma_start(out=t_bq[:, :], in_=bq[:, :])
            t_bkv = const.tile([2 * H, 1], f32)
            nc.scalar.dma_start(out=t_bkv[:, :], in_=bkv[:, :])
            t_masks = const.tile([P, 2, QW], f32)
            nc.scalar.dma_start(out=t_masks[:, :, :], in_=masks[:, :, :])
            t_id2 = const.tile([P, H], f16)
            nc.scalar.dma_start(out=t_id2[:, :], in_=id2[:, :])
            t_idf = const.tile([P, P], f16)
            nc.scalar.dma_start(out=t_idf[:, :], in_=idf[:, :])
            t_hsel = const.tile([1, 1], u32)
            nc.scalar.dma_start(out=t_hsel[:, :], in_=hsel[:, :])

            qt = big.tile([H, S], f16)           # Q^T, global column order
            kvt = big.tile([P, S], f16)          # [K^T; V^T], parity-split cols
            ktv = big.tile([P, HALF], f16)       # own half: K^T rows 0:64, V^T rows 64:128
            vones = big.tile([P, KL, H + 1], f16)
            partial = big.tile([H + 1, S], f16)  # [num; den] partials, global q order
            sum_sb = big.tile([H + 1, HALF], f16)
            osb = big.tile([P, KL, H], f32)

            ccin = dram.tile([2 * (H + 1), HALF], f16)
            ccout = dram.tile([H + 1, HALF], f16)

            # ---- Phase 1: projections (KV first, then Q interleaved with
            # V-chunk transposes; attention overlaps the Q tail) ----
            xp = None
            with (
                tc.tile_pool(name="xslab", bufs=NQ) as xp,
                tc.tile_pool(name="pkv", bufs=2, space="PSUM") as pkvp,
                tc.tile_pool(name="pq", bufs=2, space="PSUM") as pqp,
                tc.tile_pool(name="psc", bufs=3, space="PSUM") as pscp,
                tc.tile_pool(name="pav", bufs=1, space="PSUM") as pavp,
                tc.tile_pool(name="expp", bufs=KL + 2) as expp,
            ):
                slabs = []
                for n in range(NQ):
                    slab = xp.tile([P, DSL, QW], f16)
                    nc.scalar.dma_start(
                        out=slab[:, :, :],
                        in_=xts[:, QW * n : QW * (n + 1)].rearrange(
                            "(dsl p) c -> p dsl c", p=P
                        ),
                    )
                    slabs.append(slab)
                    pkv = pkvp.tile([P, QW], f32)
                    for dsl in range(DSL):
                        nc.tensor.matmul(
                            pkv[:, :],
                            t_wkv[:, dsl, :],
                            slab[:, dsl, :],
                            start=(dsl == 0),
                            stop=(dsl == DSL - 1),
                        )
                    nc.vector.tensor_scalar_add(
                        kvt[:, QW * n : QW * (n + 1)], pkv[:, :], t_bkv[:, :]
                    )

                # select own K/V half (register-offset DMA)
                with tc.tile_critical():
                    with (
                        nc.gpsimd.register("rh") as rh,
                        nc.semaphore("dynsem") as dynsem,
                    ):
                        nc.gpsimd.reg_load(rh, t_hsel[0:1, 0:1])
                        nc.gpsimd.dma_start(
                            out=ktv[:, :],
                            in_=bass.AP(kvt.tensor, rh, [[S, P], [1, HALF]]),
                        ).then_inc(dynsem, 16)
                        nc.gpsimd.wait_ge(dynsem, 16)

                # V chunks -> [k, 64] via matmul against identity, and Q
                # projections ordered so early q-slots unblock first
                nc.scalar.dma_start(out=vones[:, :, H : H + 1], in_=ones[:, :])
                qorder = [0, 4, 1, 5, 2, 6, 3, 7]
                for j in range(KL):
                    ptv = pscp.tile([P, H], f32, tag="psc")
                    nc.tensor.matmul(
                        ptv[:, :],
                        ktv[H:P, P * j : P * (j + 1)],
                        t_id2[H:P, :],
                        start=True,
                        stop=True,
                        tile_position=(H, 0),
                    )
                    nc.vector.tensor_copy(vones[:, j, 0:H], ptv[:, :])
                    if j % 2 == 0:
                        n = qorder[j // 2]
                        pq = pqp.tile([H, QW], f32)
                        for dsl in range(DSL):
                            nc.tensor.matmul(
                                pq[:, :],
                                t_wq[:, dsl, :],
                                slabs[n][:, dsl, :],
                                start=(dsl == 0),
                                stop=(dsl == DSL - 1),
                            )
                        par, i4 = n // 4, n % 4
                        qt_view = qt[:, :].rearrange(
                            "hh (ib two c) -> hh ib two c", two=2, c=P
                        )[:, 4 * i4 : 4 * i4 + 4, par, :]
                        nc.vector.tensor_scalar_add(qt_view, pq[:, :], t_bq[:, :])

                # ---- attention: scores batched ahead of the AV chain ----
                for s in range(NQ):
                    ns = 2 * s + 2
                    pav = pavp.tile([H + 1, QW], f32)
                    exs = []
                    for j in range(ns):
                        psc = pscp.tile([P, QW], f32, tag="psc")
                        nc.tensor.matmul(
                            psc[:, :],
                            ktv[0:H, P * j : P * (j + 1)],
                            qt[:, QW * s : QW * (s + 1)],
                            start=True,
                            stop=True,
                        )
                        if j >= ns - 2:
                            nc.vector.tensor_add(
                                psc[:, :], psc[:, :], t_masks[:, j - (ns - 2), :]
                            )
                        ex = expp.tile([P, QW], f16)
                        nc.scalar.activation(
                            ex[:, :], psc[:, :], Act.Exp, scale=0.125
                        )
                        exs.append(ex)
                    for j in range(ns):
                        nc.tensor.matmul(
                            pav[:, :],
                            vones[:, j, :],
                            exs[j][:, :],
                            start=(j == 0),
                            stop=(j == ns - 1),
                        )
                    nc.vector.tensor_scalar(
                        partial[:, QW * s : QW * (s + 1)], pav[:, :],
                        1.0 / 64.0, None, Alu.mult,
                    )

            # ---- Phase 4: pairwise combine (ReduceScatter add) ----
            nc.gpsimd.dma_start(out=ccin[0 : H + 1, :], in_=partial[:, 0:HALF])
            nc.gpsimd.dma_start(
                out=ccin[H + 1 : 2 * (H + 1), :], in_=partial[:, HALF:S]
            )
            nc.gpsimd.collective_compute(
                "ReduceScatter",
                Alu.add,
                replica_groups=GROUPS,
                ins=[ccin[:, :].opt()],
                outs=[ccout[:, :].opt()],
            )
            nc.gpsimd.dma_start(out=sum_sb[:, :], in_=ccout[:, :])

            # ---- Phase 5: finalize (transpose, divide, store) ----
            with (
                tc.tile_pool(name="pfin", bufs=4, space="PSUM") as pfinp,
                tc.tile_pool(name="recp", bufs=4) as recp,
            ):
                for i in range(KL):
                    pf = pfinp.tile([P, H + 1], f32)
                    nc.tensor.matmul(
                        pf[:, :],
                        sum_sb[:, P * i : P * (i + 1)],
                        t_idf[0 : H + 1, 0 : H + 1],
                        start=True,
                        stop=True,
                    )
                    rec = recp.tile([P, 1], f32)
                    nc.vector.reciprocal(rec[:, :], pf[:, H : H + 1])
                    nc.vector.tensor_scalar_mul(osb[:, i, :], pf[:, 0:H], rec[:, :])
                nc.scalar.dma_start(
                    out=out[:, :].rearrange("(i p) hh -> p i hh", p=P),
                    in_=osb[:, :, :],
                )

    nc.compile()
    return nc


_NC = None


def _get_nc():
    global _NC
    if _NC is None:
        _NC = build_nc()
    return _NC


def _mask_pattern(d):
    """[128, 512] additive causal mask: 0 where (p + d) <= j else -1e30."""
    p = np.arange(P)[:, None]
    j = np.arange(QW)[None, :]
    return np.where(p + d <= j, 0.0, NEG).astype(np.float32)


def _prepare_inputs(x, W_q, b_q, W_k, b_k, W_v, b_v):
    x = np.asarray(x, dtype=np.float32)
    W_q = np.asarray(W_q, dtype=np.float32)
    W_k = np.asarray(W_k, dtype=np.float32)
    W_v = np.asarray(W_v, dtype=np.float32)
    b_q = np.asarray(b_q, dtype=np.float32)
    b_k = np.asarray(b_k, dtype=np.float32)
    b_v = np.asarray(b_v, dtype=np.float32)

    wkv = np.ascontiguousarray(
        np.concatenate([W_k, W_v], axis=1).astype(np.float16)
    )
    wq16 = np.ascontiguousarray(W_q.astype(np.float16))
    bqc = np.ascontiguousarray(b_q.reshape(H, 1))
    bkvc = np.ascontiguousarray(np.concatenate([b_k, b_v]).reshape(2 * H, 1))
    id2 = np.zeros((P, H), np.float16)
    id2[0:H] = np.eye(H, dtype=np.float16)
    id2[H:P] = np.eye(H, dtype=np.float16)
    idf = np.eye(P, dtype=np.float16)
    ones = np.ones((P, KL), np.float16)

    xts_b = []
    for b in range(B):
        blocks = x[b].reshape(S // P, P, DM)
        xp = np.concatenate([blocks[0::2], blocks[1::2]], axis=0).reshape(S, DM)
        xts_b.append(np.ascontiguousarray(xp.T.astype(np.float16)))

    in_maps = []
    for c in range(8):
        b, h = c // 2, c % 2
        in_maps.append(
            dict(
                xts=xts_b[b],
                wq=wq16,
                wkv=wkv,
                bq=bqc,
                bkv=bkvc,
                masks=np.ascontiguousarray(
                    np.stack(
                        [_mask_pattern(P * h), _mask_pattern(256 + P * h)], axis=1
                    )
                ),
                id2=id2,
                idf=idf,
                ones=ones,
                hsel=np.array([[h * HALF]], dtype=np.uint32),
            )
        )
    return in_maps


def run_kernel(trace=False, trace_cores=None, **inputs):
    nc = _get_nc()
    in_maps = _prepare_inputs(**inputs)
    kw = {}
    if trace:
        kw["trace"] = True
        kw["trace_cores"] = trace_cores if trace_cores is not None else [0]
    res = run_bass_kernel_spmd(nc, in_maps, core_ids=list(range(8)), **kw)
    out = np.empty((B, S, H), dtype=np.float32)
    for c in range(8):
        b, h = c // 2, c % 2
        out[b, HALF * h : HALF * (h + 1), :] = res.results[c]["out"]
    return out, res


def kernel(**inputs):
    out, _ = run_kernel(**inputs)
    return out


# revision 8
# speedup vs baseline: 1.0236x; 1.0236x over previous
"""Causal single-head self-attention on 8 Trainium2 NeuronCores.

Problem: x [4, 4096, 1024], W_q/W_k/W_v [1024, 64] (+biases) ->
softmax(causal(QK^T / 8)) @ V, output [4, 4096, 64] fp32.

Sharding: data-parallel over batch (2 cores per batch). Within a pair,
k-tiles (128 rows) are parity-striped: core h in {0,1} owns k-tiles
{h, h+2, h+4, ...} of its batch. Softmax uses the no-max formulation
(scores are O(1) here), so per-core partial numerators/denominators are
linear and combine with a pairwise ReduceScatter(add). Every core runs
the IDENTICAL program (single SPMD NEFF); per-core differences (batch,
parity) are carried entirely by input data:
  - xts: x[b]^T with columns parity-split ([parity-0 k-tiles | parity-1])
    - shared by both cores of a pair
  - masks: causal-edge masks baked for the core's parity
  - hsel: register offset (h*2048) selecting the core's K/V half

Compute layout (per core):
  - All matmul operands are fp16 (full PE rate + fast weight load);
    accumulation stays fp32 in PSUM, softmax partials stay fp32.
  - Projections contract D=1024 on partitions: Q^T/[K|V]^T tiles come out
    of the PE as [features, rows].
  - Scores are computed transposed, S^T[k, q], so the softmax sum over k
    and the A@V contraction both sit on the partition axis and reduce via
    matmuls. The ones column folded into [V | 1] yields the softmax
    denominator in the same accumulation.
  - Causal masking: additive -1e30 on the two diagonal-crossing k-tiles
    per q-slot (DVE add on PSUM before the exp).
  - Transposes (V chunks, finalize) are normal-mode matmuls against an
    identity rhs: out = lhsT.T @ I.
  - Finalize: transpose [65, 128] chunks of the pair-reduced partials,
    DVE reciprocal of the denominator row, tensor_scalar multiply,
    DMA out in [q, 64] layout.
"""

import numpy as np

import concourse.bass as bass
import concourse.mybir as mybir
import concourse.tile as tile
from concourse import bacc
from concourse.bass_utils import run_bass_kernel_spmd

B, S, DM, H = 4, 4096, 1024, 64
P = 128
QW = 512            # q-tile width
NQ = S // QW        # 8 q-slots
DSL = DM // P       # 8 d-slices
KL = 16             # local (own-parity) k-tiles of 128
HALF = S // 2       # 2048
GROUPS = [[0, 1], [2, 3], [4, 5], [6, 7]]

f32 = mybir.dt.float32
f16 = mybir.dt.float16
u32 = mybir.dt.uint32
Act = mybir.ActivationFunctionType
Alu = mybir.AluOpType
NEG = -1.0e30


def build_nc():
    nc = bacc.Bacc("TRN2", target_bir_lowering=False, debug=False, num_devices=8)

    xts = nc.declare_dram_parameter("xts", [DM, S], f16, isOutput=False)
    wq = nc.declare_dram_parameter("wq", [DM, H], f16, isOutput=False)
    wkv = nc.declare_dram_parameter("wkv", [DM, 2 * H], f16, isOutput=False)
    bq = nc.declare_dram_parameter("bq", [H, 1], f32, isOutput=False)
    bkv = nc.declare_dram_parameter("bkv", [2 * H, 1], f32, isOutput=False)
    masks = nc.declare_dram_parameter("masks", [P, 2, QW], f32, isOutput=False)
    id2 = nc.declare_dram_parameter("id2", [P, H], f16, isOutput=False)
    idf = nc.declare_dram_parameter("idf", [P, P], f16, isOutput=False)
    ones = nc.declare_dram_parameter("ones", [P, KL], f16, isOutput=False)
    hsel = nc.declare_dram_parameter("hsel", [1, 1], u32, isOutput=False)
    out = nc.declare_dram_parameter("out", [HALF, H], f32, isOutput=True)

    with tile.TileContext(nc) as tc:
        with (
            tc.tile_pool(name="const", bufs=1) as const,
            tc.tile_pool(name="big", bufs=1) as big,
            tc.tile_pool(name="dram", bufs=1, space="DRAM") as dram,
        ):
            t_wq = const.tile([P, DSL, H], f16)
            nc.gpsimd.dma_start(
                out=t_wq[:, :, :],
                in_=wq[:, :].rearrange("(dsl p) hh -> p dsl hh", p=P),
            )
            t_wkv = const.tile([P, DSL, 2 * H], f16)
            nc.gpsimd.dma_start(
                out=t_wkv[:, :, :],
                in_=wkv[:, :].rearrange("(dsl p) hh -> p dsl hh", p=P),
            )
            t_bq = const.tile([H, 1], f32)
            nc.gpsimd.dma_start(out=t_bq[:, :], in_=bq[:, :])
            t_bkv = const.tile([2 * H, 1], f32)
            nc.gpsimd.dma_start(out=t_bkv[:, :], in_=bkv[:, :])
            t_masks = const.tile([P, 2, QW], f32)
            nc.gpsimd.dma_start(out=t_masks[:, :, :], in_=masks[:, :, :])
            t_id2 = const.tile([P, H], f16)
            nc.gpsimd.dma_start(out=t_id2[:, :], in_=id2[:, :])
            t_idf = const.tile([P, P], f16)
            nc.gpsimd.dma_start(out=t_idf[:, :], in_=idf[:, :])
            t_hsel = const.tile([1, 1], u32)
            nc.gpsimd.dma_start(out=t_hsel[:, :], in_=hsel[:, :])

            qt = big.tile([H, S], f16)           # Q^T, global column order
            kvt = big.tile([P, S], f16)          # [K^T; V^T], parity-split cols
            ktv = big.tile([P, HALF], f16)       # own half: K^T rows 0:64, V^T rows 64:128
            vones = big.tile([P, KL, H + 1], f16)
            partial = big.tile([H + 1, S], f16)  # [num; den] partials, global q order
            sum_sb = big.tile([H + 1, HALF], f16)
            osb = big.tile([P, KL, H], f32)

            ccin = dram.tile([2 * (H + 1), HALF], f16)
            ccout = dram.tile([H + 1, HALF], f16)

            # ---- Phase 1: projections (KV first, then Q interleaved with
            # V-chunk transposes; attention overlaps the Q tail) ----
            xp = None
            with (
                tc.tile_pool(name="xslab", bufs=NQ) as xp,
                tc.tile_pool(name="pkv", bufs=2, space="PSUM") as pkvp,
                tc.tile_pool(name="pq", bufs=2, space="PSUM") as pqp,
                tc.tile_pool(name="psc", bufs=3, space="PSUM") as pscp,
                tc.tile_pool(name="pav", bufs=1, space="PSUM") as pavp,
                tc.tile_pool(name="expp", bufs=KL + 2) as expp,
            ):
                slabs = []
                for n in range(NQ):
                    slab = xp.tile([P, DSL, QW], f16)
                    (nc.scalar if n < 2 else nc.sync).dma_start(
                        out=slab[:, :, :],
                        in_=xts[:, QW * n : QW * (n + 1)].rearrange(
                            "(dsl p) c -> p dsl c", p=P
                        ),
                    )
                    slabs.append(slab)
                    pkv = pkvp.tile([P, QW], f32)
                    for dsl in range(DSL):
                        nc.tensor.matmul(
                            pkv[:, :],
                            t_wkv[:, dsl, :],
                            slab[:, dsl, :],
                            start=(dsl == 0),
                            stop=(dsl == DSL - 1),
                        )
                    nc.vector.tensor_scalar_add(
                        kvt[:, QW * n : QW * (n + 1)], pkv[:, :], t_bkv[:, :]
                    )

                # select own K/V half (register-offset DMA); Q-projs below
                # keep the PE busy while this runs
                with tc.tile_critical():
                    with (
                        nc.gpsimd.register("rh") as rh,
                        nc.semaphore("dynsem") as dynsem,
                    ):
                        nc.gpsimd.reg_load(rh, t_hsel[0:1, 0:1])
                        nc.gpsimd.dma_start(
                            out=ktv[:, :],
                            in_=bass.AP(kvt.tensor, rh, [[S, P], [1, HALF]]),
                        ).then_inc(dynsem, 16)
                        nc.gpsimd.wait_ge(dynsem, 16)

                # Q projections, ordered so early q-slots unblock first
                qorder = [0, 4, 1, 5, 2, 6, 3, 7]
                for n in qorder:
                    pq = pqp.tile([H, QW], f32)
                    for dsl in range(DSL):
                        nc.tensor.matmul(
                            pq[:, :],
                            t_wq[:, dsl, :],
                            slabs[n][:, dsl, :],
                            start=(dsl == 0),
                            stop=(dsl == DSL - 1),
                        )
                    par, i4 = n // 4, n % 4
                    qt_view = qt[:, :].rearrange(
                        "hh (ib two c) -> hh ib two c", two=2, c=P
                    )[:, 4 * i4 : 4 * i4 + 4, par, :]
                    nc.vector.tensor_scalar_add(qt_view, pq[:, :], t_bq[:, :])

                # V chunks -> [k, 64] via matmul against identity
                nc.sync.dma_start(out=vones[:, :, H : H + 1], in_=ones[:, :])
                for j in range(KL):
                    ptv = pscp.tile([P, H], f32, tag="psc")
                    nc.tensor.matmul(
                        ptv[:, :],
                        ktv[H:P, P * j : P * (j + 1)],
                        t_id2[H:P, :],
                        start=True,
                        stop=True,
                        tile_position=(H, 0),
                    )
                    nc.vector.tensor_copy(vones[:, j, 0:H], ptv[:, :])

                # ---- attention: scores batched ahead of the AV chain;
                # per-slot partial upload for the split ReduceScatter ----
                for s in range(NQ):
                    ns = 2 * s + 2
                    pav = pavp.tile([H + 1, QW], f32)
                    exs = []
                    for j in range(ns):
                        psc = pscp.tile([P, QW], f32, tag="psc")
                        nc.tensor.matmul(
                            psc[:, :],
                            ktv[0:H, P * j : P * (j + 1)],
                            qt[:, QW * s : QW * (s + 1)],
                            start=True,
                            stop=True,
                        )
                        if j >= ns - 2:
                            nc.vector.tensor_add(
                                psc[:, :], psc[:, :], t_masks[:, j - (ns - 2), :]
                            )
                        ex = expp.tile([P, QW], f16)
                        nc.scalar.activation(
                            ex[:, :], psc[:, :], Act.Exp, scale=0.125
                        )
                        exs.append(ex)
                    for j in range(ns):
                        nc.tensor.matmul(
                            pav[:, :],
                            vones[:, j, :],
                            exs[j][:, :],
                            start=(j == 0),
                            stop=(j == ns - 1),
                        )
                    nc.vector.tensor_scalar(
                        partial[:, QW * s : QW * (s + 1)], pav[:, :],
                        1.0 / 64.0, None, Alu.mult,
                    )

            # ---- Phase 4: pairwise combine (ReduceScatter add) ----
            nc.gpsimd.dma_start(out=ccin[0 : H + 1, :], in_=partial[:, 0:HALF])
            nc.gpsimd.dma_start(
                out=ccin[H + 1 : 2 * (H + 1), :], in_=partial[:, HALF:S]
            )
            nc.gpsimd.collective_compute(
                "ReduceScatter",
                Alu.add,
                replica_groups=GROUPS,
                ins=[ccin[:, :].opt()],
                outs=[ccout[:, :].opt()],
            )
            nc.gpsimd.dma_start(out=sum_sb[:, :], in_=ccout[:, :])

            # ---- Phase 5: finalize (transpose, divide, store) ----
            with (
                tc.tile_pool(name="pfin", bufs=4, space="PSUM") as pfinp,
                tc.tile_pool(name="recp", bufs=4) as recp,
            ):
                for i in range(KL):
                    pf = pfinp.tile([P, H + 1], f32)
                    nc.tensor.matmul(
                        pf[:, :],
                        sum_sb[:, P * i : P * (i + 1)],
                        t_idf[0 : H + 1, 0 : H + 1],
                        start=True,
                        stop=True,
                    )
                    rec = recp.tile([P, 1], f32)
                    nc.vector.reciprocal(rec[:, :], pf[:, H : H + 1])
                    nc.vector.tensor_scalar_mul(osb[:, i, :], pf[:, 0:H], rec[:, :])
                nc.sync.dma_start(
                    out=out[:, :].rearrange("(i p) hh -> p i hh", p=P),
                    in_=osb[:, :, :],
                )

    nc.compile()
    return nc


_NC = None


def _get_nc():
    global _NC
    if _NC is None:
        _NC = build_nc()
    return _NC


def _mask_pattern(d):
    """[128, 512] additive causal mask: 0 where (p + d) <= j else -1e30."""
    p = np.arange(P)[:, None]
    j = np.arange(QW)[None, :]
    return np.where(p + d <= j, 0.0, NEG).astype(np.float32)


def _prepare_inputs(x, W_q, b_q, W_k, b_k, W_v, b_v):
    x = np.asarray(x, dtype=np.float32)
    W_q = np.asarray(W_q, dtype=np.float32)
    W_k = np.asarray(W_k, dtype=np.float32)
    W_v = np.asarray(W_v, dtype=np.float32)
    b_q = np.asarray(b_q, dtype=np.float32)
    b_k = np.asarray(b_k, dtype=np.float32)
    b_v = np.asarray(b_v, dtype=np.float32)

    wkv = np.ascontiguousarray(
        np.concatenate([W_k, W_v], axis=1).astype(np.float16)
    )
    wq16 = np.ascontiguousarray(W_q.astype(np.float16))
    bqc = np.ascontiguousarray(b_q.reshape(H, 1))
    bkvc = np.ascontiguousarray(np.concatenate([b_k, b_v]).reshape(2 * H, 1))
    id2 = np.zeros((P, H), np.float16)
    id2[0:H] = np.eye(H, dtype=np.float16)
    id2[H:P] = np.eye(H, dtype=np.float16)
    idf = np.eye(P, dtype=np.float16)
    ones = np.ones((P, KL), np.float16)

    xts_b = []
    for b in range(B):
        blocks = x[b].reshape(S // P, P, DM)
        xp = np.concatenate([blocks[0::2], blocks[1::2]], axis=0).reshape(S, DM)
        xts_b.append(np.ascontiguousarray(xp.T.astype(np.float16)))

    in_maps = []
    for c in range(8):
        b, h = c // 2, c % 2
        in_maps.append(
            dict(
                xts=xts_b[b],
                wq=wq16,
                wkv=wkv,
                bq=bqc,
                bkv=bkvc,
                masks=np.ascontiguousarray(
                    np.stack(
                        [_mask_pattern(P * h), _mask_pattern(256 + P * h)], axis=1
                    )
                ),
                id2=id2,
                idf=idf,
                ones=ones,
                hsel=np.array([[h * HALF]], dtype=np.uint32),
            )
        )
    return in_maps


def run_kernel(trace=False, trace_cores=None, **inputs):
    nc = _get_nc()
    in_maps = _prepare_inputs(**inputs)
    kw = {}
    if trace:
        kw["trace"] = True
        kw["trace_cores"] = trace_cores if trace_cores is not None else [0]
    res = run_bass_kernel_spmd(nc, in_maps, core_ids=list(range(8)), **kw)
    out = np.empty((B, S, H), dtype=np.float32)
    for c in range(8):
        b, h = c // 2, c % 2
        out[b, HALF * h : HALF * (h + 1), :] = res.results[c]["out"]
    return out, res


def kernel(**inputs):
    out, _ = run_kernel(**inputs)
    return out
